# revision 35
# baseline (speedup 1.0000x reference)
"""Trainium2 Bass kernel for nn_GCIQEValue (MLP + IQE head), 8-core data parallel.

Math (validated vs reference):
  phi(x) = LN-MLP: 3x [matmul+bias -> tanh-gelu -> LayerNorm(affine folded into
  next W on host)] then final matmul+bias.
  IQE per row, per 32-dim component c with x = phi_s[c], y = phi_g[c]:
    y' = max(x, y)                      (interval [x_i, max(x_i,y_i)])
    u = sort(x), v = sort(y')           (independent keys-only sorts)
    comp_c = sum(v) - u_0 - sum_{i>=1} max(u_i, v_{i-1})
  out = sig(alpha) * mean_c(comp) + (1 - sig(alpha)) * max_c(comp)

v3: fp16 weights/activations/transposes (PE fp32 matmul runs at 1/4 rate),
fp16 bitonic sort (DVE 2x mode on aligned shift passes), and T=2 row-tiles
per pipeline iteration so every DVE/ACT fixed cost amortizes over 256 rows.
When the effective biases are zero (structurally true for this generator)
the LayerNorm affine folds forward: LN(g) @ W == rstd * (g@W - m*colsum(W)),
so the per-row scale rides the next layer's gelu `scale` operand and the
-m*colsum(W) term is one K=1 matmul row per (stream, tile); the four -m rows
are transposed into one partition-0 PSUM strip and copied once. LN stats
are batched across streams and tiles (one [P,4] Newton-rsqrt chain/layer).

Structure: 7-stage software pipeline (For_i_pipelined) over 256-row tiles:
  S0 load | S1 L0 | S2 L1 | S3 L2 | S4 L3+ymax | S5 sort p0-6 | S6 sort p7-14+post
"""

import numpy as np

B = 131072
OBS = 64
H = 512
NCOMP = 16
DPC = 32
NCORES = 8
P = 128
LN_EPS = 1e-6

_CACHE = {}

# bitonic schedule for 32-wide ascending sort: 15 passes
_SCHED = [("pair", 0, 0)]
for _L in (4, 8, 16, 32):
    _SCHED.append(("flip", _L, 0))
    _d = _L // 4
    while _d >= 1:
        _SCHED.append(("shift", _L, _d))
        _d //= 2


# ---------------------------------------------------------------- device kernel
def build_nc(rows_per_core=B // NCORES, unroll=4, repeats=1, tiles=2,
             stage_bufs=None, mlp_bufs=3, pz_bufs=0, split_pass=7,
             n_passes=15, newton=2, sort16=1, mm16=1, has_bias=0,
             n_layers=3, ln_lite=False, hints=False):
    """Build the Bass (Bacc) module for one core processing rows_per_core rows.

    has_bias=0 uses the folded-LN fast path (valid only when effective biases
    are zero); has_bias=1 keeps a standalone scale-apply ACT op per stream.
    """
    import concourse.bass as bass
    import concourse.mybir as mybir
    import concourse.tile as tile
    from concourse import bacc
    from concourse.masks import make_identity

    fp32 = mybir.dt.float32
    i32 = mybir.dt.int32
    adt = mybir.dt.float16 if mm16 else fp32   # activations/weights
    sdt = mybir.dt.float16 if sort16 else fp32  # sort buffers
    AT = mybir.ActivationFunctionType
    OP = mybir.AluOpType
    fold = (not has_bias) and mm16  # folded path needs the fp16 bank layout
    T = tiles
    S2 = 2 * T          # stat columns: (stream, tile)

    nt = rows_per_core // (T * P)
    assert rows_per_core % (T * P) == 0
    if stage_bufs is None:
        stage_bufs = unroll
    if pz_bufs == 0:
        pz_bufs = 4 // T

    nc = bacc.Bacc("TRN2", target_bir_lowering=False, debug=False)

    obs = nc.declare_dram_parameter("observations", [rows_per_core, OBS], adt,
                                    isOutput=False)
    gls = nc.declare_dram_parameter("goals", [rows_per_core, OBS], adt,
                                    isOutput=False)
    w0d = nc.declare_dram_parameter("w0", [OBS, H], adt, isOutput=False)
    w1d = nc.declare_dram_parameter("w1", [H, H], adt, isOutput=False)
    w2d = nc.declare_dram_parameter("w2", [H, H], adt, isOutput=False)
    w3d = nc.declare_dram_parameter("w3", [H, H], adt, isOutput=False)
    cwd = nc.declare_dram_parameter("csw", [3, H], adt, isOutput=False)
    if has_bias:
        bsd = nc.declare_dram_parameter("bs", [4, H], adt, isOutput=False)
    avd = nc.declare_dram_parameter("avec", [P, 2], fp32, isOutput=False)
    out = nc.declare_dram_parameter("out", [rows_per_core], fp32, isOutput=True)

    obs_v = obs[:].rearrange("(n t p) f -> n p t f", t=T, p=P)
    gls_v = gls[:].rearrange("(n t p) f -> n p t f", t=T, p=P)
    out_v = out[:].rearrange("(n t p) -> n p t", t=T, p=P)

    with tile.TileContext(nc) as tc:
        with (
            tc.tile_pool(name="const", bufs=1) as cpool,
            tc.tile_pool(name="mlp", bufs=mlp_bufs) as mp,
            tc.tile_pool(name="srt", bufs=mlp_bufs) as sp,
            tc.tile_pool(name="pipe", bufs=1) as pipe_pool,
            tc.tile_pool(name="ps", bufs=pz_bufs, space="PSUM") as pp,
            tc.tile_pool(name="pst", bufs=2, space="PSUM") as ppt,
            tc.tile_pool(name="psm", bufs=2, space="PSUM") as ppm,
        ):
            # ---- constants
            w0 = cpool.tile([OBS, H], adt)
            nc.sync.dma_start(out=w0, in_=w0d[:])
            wl = []
            for wd, nm in ((w1d, "w1"), (w2d, "w2"), (w3d, "w3")):
                t = cpool.tile([P, 4, H], adt, tag=nm)
                nc.sync.dma_start(out=t, in_=wd[:].rearrange("(c p) n -> p c n", p=P))
                wl.append(t)
            csw = cpool.tile([1, 3, H], adt)
            nc.sync.dma_start(out=csw,
                              in_=cwd[:].rearrange("(o c) n -> o c n", o=1))
            if has_bias:
                bsc = cpool.tile([1, 4, H], adt)
                nc.sync.dma_start(out=bsc, in_=bsd[:].rearrange("(o c) n -> o c n", o=1))
                ones = cpool.tile([1, P], adt)
                nc.vector.memset(ones, 1.0)
            avec = cpool.tile([P, 2], fp32)
            nc.sync.dma_start(out=avec, in_=avd[:])
            ident = cpool.tile([P, P], adt)
            make_identity(nc, ident)

            def matmuls(src, li, negmT, s, pTf):
                """src [P, T, F_in] adt (stream s) -> pz PSUM fp32 [P, T, 512].
                negmT: [1, S2*P] SBUF strip of -m rows (folded path)."""
                pz = pp.tile([P, T, H], fp32, tag="pz")
                if li == 0:
                    for t in range(T):
                        nc.tensor.transpose(pTf[0:OBS, t * P:(t + 1) * P],
                                            src[:, t, :], ident)
                    xT = mp.tile([OBS, T, P], adt, tag="xT")
                    nc.scalar.copy(xT, pTf[0:OBS, 0:T * P])
                    for t in range(T):
                        nc.tensor.matmul(pz[:, t, :], xT[:, t, :], w0,
                                         start=True, stop=(not has_bias))
                else:
                    for t in range(T):
                        for k in range(4):
                            nc.tensor.transpose(
                                pTf[:, t * H + k * P: t * H + (k + 1) * P],
                                src[:, t, k * P:(k + 1) * P], ident)
                    tT = mp.tile([P, T, 4, P], adt, tag="tT")
                    nc.scalar.copy(tT, pTf[:, 0:T * H])
                    plain_stop = (not has_bias) and negmT is None
                    for t in range(T):
                        for k in range(4):
                            nc.tensor.matmul(pz[:, t, :], tT[:, t, k, :],
                                             wl[li - 1][:, k, :],
                                             start=(k == 0),
                                             stop=(plain_stop and k == 3))
                        if negmT is not None:
                            c = (s * T + t) * P
                            nc.tensor.matmul(pz[:, t, :],
                                             negmT[0:1, c:c + P],
                                             csw[0:1, li - 1, :],
                                             start=False, stop=(not has_bias))
                if has_bias:
                    for t in range(T):
                        nc.tensor.matmul(pz[:, t, :], ones, bsc[:, li, :],
                                         start=False, stop=True)
                return pz

            def ln_stats(g, sums, pz_o, pz_g, pmt):
                """Row stats of raw-gelu g [P,2,T,H] (gelu sums in `sums`
                [P,S2], column c = s*T+t): returns rstd [P,S2] fp32; the
                folded path leaves the -m rows transposed in pmt[0:1, :].
                Non-fold path rescales g in place instead."""
                sq = mp.tile([P, S2], fp32, tag="sq")
                for s in range(2):
                    for t in range(T):
                        c = s * T + t
                        # scratch target: keeps the pz PSUM slot free after gelu
                        gsq = mp.tile([P, H], adt, tag="gsq")
                        nc.scalar.activation(gsq, g[:, s, t, :], AT.Square,
                                             accum_out=sq[:, c:c + 1])
                nc.vector.tensor_scalar_mul(sq, sq, 1.0 / H)
                m = mp.tile([P, S2], fp32, tag="m")
                mf = m
                nc.vector.tensor_scalar_mul(mf, sums, 1.0 / H)
                varb = mp.tile([P, S2], fp32, tag="varb")
                nc.vector.tensor_tensor(out=varb, in0=mf, in1=mf, op=OP.mult)
                nc.vector.tensor_tensor(out=varb, in0=sq, in1=varb,
                                        op=OP.subtract)
                nc.vector.tensor_scalar_add(varb, varb, LN_EPS)
                # rsqrt: quake seed on int ALU + `newton` NR iterations
                yi = mp.tile([P, S2], i32, tag="yi")
                nc.vector.tensor_scalar(
                    out=yi, in0=varb.bitcast(i32), scalar1=1,
                    scalar2=None, op0=OP.logical_shift_right)
                nc.vector.tensor_scalar(
                    out=yi, in0=yi, scalar1=-1, scalar2=0x5F3759DF,
                    op0=OP.mult, op1=OP.add)
                y = yi.bitcast(fp32)
                t1 = mp.tile([P, S2], fp32, tag="nt1")
                for _ in range(newton):
                    nc.vector.tensor_tensor(out=t1, in0=varb, in1=y, op=OP.mult)
                    nc.vector.tensor_tensor(out=t1, in0=t1, in1=y, op=OP.mult)
                    nc.vector.tensor_scalar(out=t1, in0=t1, scalar1=-0.5,
                                            scalar2=1.5, op0=OP.mult,
                                            op1=OP.add)
                    nc.vector.tensor_tensor(out=y, in0=y, in1=t1, op=OP.mult)
                if fold:
                    mh = mp.tile([P, S2], adt, tag="mh")
                    nc.vector.tensor_scalar(out=mh, in0=mf, scalar1=-1.0,
                                            scalar2=None, op0=OP.mult)
                    # all S2 -m rows transposed onto one partition-0 strip
                    for c in range(S2):
                        nc.tensor.transpose(pmt[0:1, c * P:(c + 1) * P],
                                            mh[:, c:c + 1], ident)
                    return y
                nmr = mp.tile([P, S2], fp32, tag="nmr")
                nc.vector.scalar_tensor_tensor(out=nmr, in0=mf, scalar=-1.0,
                                               in1=y, op0=OP.mult, op1=OP.mult)
                for s in range(2):
                    for t in range(T):
                        c = s * T + t
                        nc.scalar.activation(g[:, s, t, :], g[:, s, t, :],
                                             AT.Identity,
                                             bias=nmr[:, c:c + 1],
                                             scale=y[:, c:c + 1])
                return y

            def emit_sort_pass(p_idx, src_x, src_y, dst):
                """Emit bitonic pass p_idx over all T row-tiles. Pass 0 reads
                (src_x, src_y) [P,T,H]; later passes read src_x [P,T*1024]."""
                kind, L, d = _SCHED[p_idx]
                V = nc.vector
                if kind == "pair":
                    dv = dst.rearrange("p (t h g e) -> p t h g e",
                                       t=T, h=2, e=DPC)
                    for src, hh in ((src_x, 0), (src_y, 1)):
                        s = src.rearrange("p t (g e) -> p t g e", e=DPC)
                        V.tensor_tensor(out=dv[:, :, hh, :, 0::2],
                                        in0=s[:, :, :, 0::2],
                                        in1=s[:, :, :, 1::2], op=OP.min)
                        V.tensor_tensor(out=dv[:, :, hh, :, 1::2],
                                        in0=s[:, :, :, 0::2],
                                        in1=s[:, :, :, 1::2], op=OP.max)
                elif kind == "flip":
                    half = L // 2
                    s = src_x.rearrange("p (b e) -> p b e", e=L)
                    o = dst.rearrange("p (b e) -> p b e", e=L)
                    V.tensor_tensor(out=o[:, :, 0:half], in0=s[:, :, 0:half],
                                    in1=s[:, :, L - 1:half - 1:-1], op=OP.min)
                    V.tensor_tensor(out=o[:, :, half:L], in0=s[:, :, half:L],
                                    in1=s[:, :, half - 1::-1], op=OP.max)
                else:
                    s = src_x.rearrange("p (c e) -> p c e", e=2 * d)
                    o = dst.rearrange("p (c e) -> p c e", e=2 * d)
                    V.tensor_tensor(out=o[:, :, 0:d], in0=s[:, :, 0:d],
                                    in1=s[:, :, d:2 * d], op=OP.min)
                    V.tensor_tensor(out=o[:, :, d:2 * d], in0=s[:, :, 0:d],
                                    in1=s[:, :, d:2 * d], op=OP.max)

            # ---------------- pipeline stages
            def st_load(pipe, iv):
                xt = pipe.intermediate_tile([P, T, OBS], adt, name="xt")
                gt = pipe.intermediate_tile([P, T, OBS], adt, name="gt")
                nc.sync.dma_start(out=xt, in_=obs_v[iv])
                nc.sync.dma_start(out=gt, in_=gls_v[iv])
                return (xt, gt)

            def gelu_pair(pz_o, pz_g, g, rstd, sums):
                for s, pz in ((0, pz_o), (1, pz_g)):
                    for t in range(T):
                        c = s * T + t
                        kw = {}
                        if fold and rstd is not None:
                            kw["scale"] = rstd[:, c:c + 1]
                        nc.scalar.activation(g[:, s, t, :], pz[:, t, :],
                                             AT.Gelu_apprx_tanh,
                                             accum_out=sums[:, c:c + 1], **kw)

            def mk_layer(li):
                def st(pipe, iv, prev):
                    if li == 0:
                        srcs = prev
                        rstd_in = negmT_in = None
                    else:
                        g_in = prev[0]
                        rstd_in = prev[1] if len(prev) > 1 else None
                        negmT_in = prev[2] if len(prev) > 2 else None
                        srcs = (g_in[:, 0], g_in[:, 1])
                    g = pipe.intermediate_tile([P, 2, T, H], adt, name=f"g{li}")
                    if li >= n_layers:  # ablation: copy-through this mid layer
                        nc.scalar.copy(g, g_in)
                        return prev[:0] + (g,) + prev[1:]
                    pTf = ppt.tile([P, T * H], adt, tag="pT")
                    pz_o = matmuls(srcs[0], li, negmT_in, 0, pTf)
                    pz_g = matmuls(srcs[1], li, negmT_in, 1, pTf)
                    sums = mp.tile([P, S2], fp32, tag="sums")
                    gelu_pair(pz_o, pz_g, g, rstd_in, sums)
                    if ln_lite:
                        return (g,)
                    pmt = ppm.tile([1, S2 * P], adt, tag="pmt")
                    rstd = ln_stats(g, sums, pz_o, pz_g, pmt)
                    if fold:
                        negmT = pipe.intermediate_tile([1, S2 * P], adt,
                                                       name=f"nmT{li}")
                        nc.scalar.copy(negmT, pmt[0:1, :])
                        rs = pipe.intermediate_tile([P, S2], fp32,
                                                    name=f"rstd{li}")
                        nc.vector.tensor_copy(rs, rstd)
                        return (g, rs, negmT)
                    return (g,)
                return st

            def st_l3(pipe, iv, prev):
                g_in = prev[0]
                rstd_in = prev[1] if len(prev) > 1 else None
                negmT_in = prev[2] if len(prev) > 2 else None
                phis = pipe.intermediate_tile([P, T, H], sdt, name="phis")
                ypr = pipe.intermediate_tile([P, T, H], sdt, name="ypr")
                pTf = ppt.tile([P, T * H], adt, tag="pT")
                pz = matmuls(g_in[:, 0], 3, negmT_in, 0, pTf)
                for t in range(T):
                    if fold and rstd_in is not None:
                        nc.scalar.mul(phis[:, t, :], pz[:, t, :],
                                      rstd_in[:, t:t + 1])
                    else:
                        nc.scalar.copy(phis[:, t, :], pz[:, t, :])
                pzg = matmuls(g_in[:, 1], 3, negmT_in, 1, pTf)
                for t in range(T):
                    if fold and rstd_in is not None:
                        nc.vector.scalar_tensor_tensor(
                            out=ypr[:, t, :], in0=pzg[:, t, :],
                            scalar=rstd_in[:, T + t:T + t + 1],
                            in1=phis[:, t, :],
                            op0=OP.mult, op1=OP.max)
                    else:
                        nc.vector.tensor_tensor(out=ypr[:, t, :],
                                                in0=phis[:, t, :],
                                                in1=pzg[:, t, :], op=OP.max)
                return (phis, ypr)

            def st_sort_a(pipe, iv, prev):
                phis, ypr = prev
                bufA = pipe.intermediate_tile([P, T * 2 * H], sdt, name="bufA")
                bufB = pipe.intermediate_tile([P, T * 2 * H], sdt, name="bufB")
                emit_sort_pass(0, phis, ypr, bufA)
                cur, nxt = bufA, bufB
                for pidx in range(1, split_pass):
                    if pidx < n_passes:
                        emit_sort_pass(pidx, cur, None, nxt)
                    cur, nxt = nxt, cur
                return (bufA, bufB)

            def st_sort_b(pipe, iv, prev):
                bufA, bufB = prev
                cur, nxt = (bufB, bufA) if split_pass % 2 == 0 else (bufA, bufB)
                for pidx in range(split_pass, 15):
                    if pidx < n_passes:
                        emit_sort_pass(pidx, cur, None, nxt)
                    cur, nxt = nxt, cur
                fin = cur
                fv = fin.rearrange("p (t h g e) -> p t h g e", t=T, h=2, e=DPC)
                # coupling: u[i] <- max(u[i], v[i-1]) for i>=1, in place
                for t in range(T):
                    nc.vector.tensor_tensor(out=fv[:, t, 0, :, 1:DPC],
                                            in0=fv[:, t, 0, :, 1:DPC],
                                            in1=fv[:, t, 1, :, 0:DPC - 1],
                                            op=OP.max)
                red = sp.tile([P, T, 2, NCOMP], fp32, tag="red")
                nc.vector.tensor_reduce(
                    out=red.rearrange("p t h g -> p (t h g)"),
                    in_=fin.rearrange("p (a e) -> p a e", e=DPC),
                    axis=mybir.AxisListType.X, op=OP.add)
                comp = sp.tile([P, T, NCOMP], fp32, tag="comp")
                nc.vector.tensor_tensor(out=comp, in0=red[:, :, 1, :],
                                        in1=red[:, :, 0, :], op=OP.subtract)
                cs = sp.tile([P, T], fp32, tag="cs")
                nc.vector.tensor_reduce(out=cs, in_=comp,
                                        axis=mybir.AxisListType.X, op=OP.add)
                cm = sp.tile([P, T], fp32, tag="cm")
                nc.vector.tensor_reduce(out=cm, in_=comp,
                                        axis=mybir.AxisListType.X, op=OP.max)
                res = sp.tile([P, T], fp32, tag="res")
                nc.vector.tensor_scalar(out=res, in0=cs, scalar1=avec[:, 0:1],
                                        scalar2=None, op0=OP.mult)
                nc.vector.scalar_tensor_tensor(out=res, in0=cm,
                                               scalar=avec[:, 1:2], in1=res,
                                               op0=OP.mult, op1=OP.add)
                nc.sync.dma_start(out=out_v[iv], in_=res)

            stages = [st_load, mk_layer(0), mk_layer(1), mk_layer(2), st_l3,
                      st_sort_a, st_sort_b]

            def run_pipe():
                he = (mybir.EngineType.PE, mybir.EngineType.DVE,
                      mybir.EngineType.Activation, mybir.EngineType.SP,
                      mybir.EngineType.Pool) if hints else ()
                tc.For_i_pipelined(stages, 0, nt, 1, pool=pipe_pool,
                                   unroll=unroll, staged_num_bufs=stage_bufs,
                                   hint_engines=he)

            if repeats == 1:
                run_pipe()
            else:
                with tc.For_i(0, repeats, 1):
                    run_pipe()

    nc.finalize()
    return nc


# ---------------------------------------------------------------- host wrapper
def _prep_host(inputs, mm16=True):
    """Fold LN affine params into the following layer's weights; build avec."""
    f32 = np.float32
    adt = np.float16 if mm16 else f32
    W0 = np.asarray(inputs["W0"], f32)
    b0 = np.asarray(inputs["b0"], f32)
    w, b = [W0], [b0]
    for i in (0, 1, 2):
        s = np.asarray(inputs[f"ln{i}_s"], f32)
        t = np.asarray(inputs[f"ln{i}_b"], f32)
        Wn = np.asarray(inputs[("W1", "W2", "W3")[i]], f32)
        bn = np.asarray(inputs[("b1", "b2", "b3")[i]], f32)
        w.append(s[:, None] * Wn)
        b.append(bn + t @ Wn)
    bs = np.stack(b, 0)  # [4, 512]
    has_bias = bool(np.any(np.abs(bs) > 0))
    csw = np.stack([w[1].sum(0), w[2].sum(0), w[3].sum(0)], 0)  # [3, 512]
    alpha = float(np.asarray(inputs["alpha"]))
    a = 1.0 / (1.0 + np.exp(-alpha))
    avec = np.empty((P, 2), f32)
    avec[:, 0] = a / NCOMP
    avec[:, 1] = 1.0 - a
    return ([x.astype(adt) for x in w], bs.astype(adt), csw.astype(adt),
            avec, has_bias)


def _probe_devices():
    """Poke every core with a tiny op; retries to shake off a stale
    NRT_EXEC_UNIT_UNRECOVERABLE state left by a previous process."""
    import jax
    import jax.numpy as jnp

    for attempt in range(3):
        try:
            for d in jax.devices()[:NCORES]:
                jnp.zeros((1,), jnp.float32, device=d).block_until_ready()
            return
        except Exception:
            if attempt == 2:
                raise


def run_on_device(inputs, rows_total=B, trace=False, repeats=1, **build_kw):
    """Shard, run on 8 cores, gather. Returns (out [rows_total], results obj)."""
    from concourse.bass_utils import run_bass_kernel_spmd

    _probe_devices()

    mm16 = bool(build_kw.get("mm16", 1))
    (w0, w1, w2, w3), bs, csw, avec, has_bias = _prep_host(inputs, mm16=mm16)
    build_kw.setdefault("has_bias", int(has_bias))

    rows_core = rows_total // NCORES
    key = (rows_core, repeats, tuple(sorted(build_kw.items())))
    if key not in _CACHE:
        _CACHE[key] = build_nc(rows_core, repeats=repeats, **build_kw)
    nc = _CACHE[key]

    adt = np.float16 if mm16 else np.float32
    ob = np.ascontiguousarray(
        np.asarray(inputs["observations"])[:rows_total].astype(adt))
    gl = np.ascontiguousarray(
        np.asarray(inputs["goals"])[:rows_total].astype(adt))
    in_maps = []
    for c in range(NCORES):
        sl = slice(c * rows_core, (c + 1) * rows_core)
        m = {
            "observations": ob[sl], "goals": gl[sl],
            "w0": w0, "w1": w1, "w2": w2, "w3": w3, "csw": csw, "avec": avec,
        }
        if has_bias:
            m["bs"] = bs
        in_maps.append(m)
    r = run_bass_kernel_spmd(nc, in_maps, list(range(NCORES)), trace=trace)
    outp = np.concatenate([r.results[c]["out"] for c in range(NCORES)])
    return outp, r


def kernel(**inputs):
    out, _ = run_on_device(inputs)
    return out.astype(np.float32)


# revision 39
# speedup vs baseline: 1.1221x; 1.1221x over previous
"""Trainium2 Bass kernel for nn_GCIQEValue (MLP + IQE head), 8-core data parallel.

Math (validated vs reference):
  phi(x) = LN-MLP: 3x [matmul+bias -> tanh-gelu -> LayerNorm(affine folded into
  next W on host)] then final matmul+bias.
  IQE per row, per 32-dim component c with x = phi_s[c], y = phi_g[c]:
    y' = max(x, y)                      (interval [x_i, max(x_i,y_i)])
    u = sort(x), v = sort(y')           (independent keys-only sorts)
    comp_c = sum(v) - u_0 - sum_{i>=1} max(u_i, v_{i-1})
  out = sig(alpha) * mean_c(comp) + (1 - sig(alpha)) * max_c(comp)

v3: fp16 weights/activations/transposes (PE fp32 matmul runs at 1/4 rate),
fp16 bitonic sort (DVE 2x mode on aligned shift passes), and T=2 row-tiles
per pipeline iteration so every DVE/ACT fixed cost amortizes over 256 rows.
When the effective biases are zero (structurally true for this generator)
the LayerNorm affine folds forward: LN(g) @ W == rstd * (g@W - m*colsum(W)),
so the per-row scale rides the next layer's gelu `scale` operand and the
-m*colsum(W) term is one K=1 matmul row per (stream, tile); the four -m rows
are transposed into one partition-0 PSUM strip and copied once. LN stats
are batched across streams and tiles (one [P,4] Newton-rsqrt chain/layer).

Structure: 7-stage software pipeline (For_i_pipelined) over 256-row tiles:
  S0 load | S1 L0 | S2 L1 | S3 L2 | S4 L3+ymax | S5 sort p0-6 | S6 sort p7-14+post
"""

import numpy as np

B = 131072
OBS = 64
H = 512
NCOMP = 16
DPC = 32
NCORES = 8
P = 128
LN_EPS = 1e-6

_CACHE = {}

# bitonic schedule for 32-wide ascending sort: 15 passes
_SCHED = [("pair", 0, 0)]
for _L in (4, 8, 16, 32):
    _SCHED.append(("flip", _L, 0))
    _d = _L // 4
    while _d >= 1:
        _SCHED.append(("shift", _L, _d))
        _d //= 2


# ---------------------------------------------------------------- device kernel
def build_nc(rows_per_core=B // NCORES, unroll=4, repeats=1, tiles=2,
             stage_bufs=None, mlp_bufs=3, pz_bufs=0, split_pass=7,
             n_passes=15, newton=2, sort16=1, mm16=1, has_bias=0,
             ln_pool=1, tail_pool=1, n_layers=3, ln_lite=False, hints=False):
    """Build the Bass (Bacc) module for one core processing rows_per_core rows.

    has_bias=0 uses the folded-LN fast path (valid only when effective biases
    are zero); has_bias=1 keeps a standalone scale-apply ACT op per stream.
    """
    import concourse.bass as bass
    import concourse.mybir as mybir
    import concourse.tile as tile
    from concourse import bacc
    from concourse.masks import make_identity

    fp32 = mybir.dt.float32
    i32 = mybir.dt.int32
    adt = mybir.dt.float16 if mm16 else fp32   # activations/weights
    sdt = mybir.dt.float16 if sort16 else fp32  # sort buffers
    AT = mybir.ActivationFunctionType
    OP = mybir.AluOpType
    fold = (not has_bias) and mm16  # folded path needs the fp16 bank layout
    T = tiles
    S2 = 2 * T          # stat columns: (stream, tile)

    nt = rows_per_core // (T * P)
    assert rows_per_core % (T * P) == 0
    if stage_bufs is None:
        stage_bufs = unroll
    if pz_bufs == 0:
        pz_bufs = 4 // T

    nc = bacc.Bacc("TRN2", target_bir_lowering=False, debug=False)

    obs = nc.declare_dram_parameter("observations", [rows_per_core, OBS], adt,
                                    isOutput=False)
    gls = nc.declare_dram_parameter("goals", [rows_per_core, OBS], adt,
                                    isOutput=False)
    w0d = nc.declare_dram_parameter("w0", [OBS, H], adt, isOutput=False)
    w1d = nc.declare_dram_parameter("w1", [H, H], adt, isOutput=False)
    w2d = nc.declare_dram_parameter("w2", [H, H], adt, isOutput=False)
    w3d = nc.declare_dram_parameter("w3", [H, H], adt, isOutput=False)
    cwd = nc.declare_dram_parameter("csw", [3, H], adt, isOutput=False)
    if has_bias:
        bsd = nc.declare_dram_parameter("bs", [4, H], adt, isOutput=False)
    avd = nc.declare_dram_parameter("avec", [P, 2], fp32, isOutput=False)
    out = nc.declare_dram_parameter("out", [rows_per_core], fp32, isOutput=True)

    obs_v = obs[:].rearrange("(n t p) f -> n p t f", t=T, p=P)
    gls_v = gls[:].rearrange("(n t p) f -> n p t f", t=T, p=P)
    out_v = out[:].rearrange("(n t p) -> n p t", t=T, p=P)

    with tile.TileContext(nc) as tc:
        with (
            tc.tile_pool(name="const", bufs=1) as cpool,
            tc.tile_pool(name="mlp", bufs=mlp_bufs) as mp,
            tc.tile_pool(name="srt", bufs=mlp_bufs) as sp,
            tc.tile_pool(name="pipe", bufs=1) as pipe_pool,
            tc.tile_pool(name="ps", bufs=pz_bufs, space="PSUM") as pp,
            tc.tile_pool(name="pst", bufs=2, space="PSUM") as ppt,
            tc.tile_pool(name="psm", bufs=2, space="PSUM") as ppm,
        ):
            # ---- constants
            w0 = cpool.tile([OBS, H], adt)
            nc.sync.dma_start(out=w0, in_=w0d[:])
            wl = []
            for wd, nm in ((w1d, "w1"), (w2d, "w2"), (w3d, "w3")):
                t = cpool.tile([P, 4, H], adt, tag=nm)
                nc.sync.dma_start(out=t, in_=wd[:].rearrange("(c p) n -> p c n", p=P))
                wl.append(t)
            csw = cpool.tile([1, 3, H], adt)
            nc.sync.dma_start(out=csw,
                              in_=cwd[:].rearrange("(o c) n -> o c n", o=1))
            if has_bias:
                bsc = cpool.tile([1, 4, H], adt)
                nc.sync.dma_start(out=bsc, in_=bsd[:].rearrange("(o c) n -> o c n", o=1))
                ones = cpool.tile([1, P], adt)
                nc.vector.memset(ones, 1.0)
            avec = cpool.tile([P, 2], fp32)
            nc.sync.dma_start(out=avec, in_=avd[:])
            ident = cpool.tile([P, P], adt)
            make_identity(nc, ident)

            def matmuls(src, li, negmT, s, pTf):
                """src [P, T, F_in] adt (stream s) -> pz PSUM fp32 [P, T, 512].
                negmT: [1, S2*P] SBUF strip of -m rows (folded path)."""
                pz = pp.tile([P, T, H], fp32, tag="pz")
                if li == 0:
                    for t in range(T):
                        nc.tensor.transpose(pTf[0:OBS, t * P:(t + 1) * P],
                                            src[:, t, :], ident)
                    xT = mp.tile([OBS, T, P], adt, tag="xT")
                    nc.scalar.copy(xT, pTf[0:OBS, 0:T * P])
                    for t in range(T):
                        nc.tensor.matmul(pz[:, t, :], xT[:, t, :], w0,
                                         start=True, stop=(not has_bias))
                else:
                    for t in range(T):
                        for k in range(4):
                            nc.tensor.transpose(
                                pTf[:, t * H + k * P: t * H + (k + 1) * P],
                                src[:, t, k * P:(k + 1) * P], ident)
                    tT = mp.tile([P, T, 4, P], adt, tag="tT")
                    nc.scalar.copy(tT, pTf[:, 0:T * H])
                    plain_stop = (not has_bias) and negmT is None
                    for t in range(T):
                        for k in range(4):
                            nc.tensor.matmul(pz[:, t, :], tT[:, t, k, :],
                                             wl[li - 1][:, k, :],
                                             start=(k == 0),
                                             stop=(plain_stop and k == 3))
                        if negmT is not None:
                            c = (s * T + t) * P
                            nc.tensor.matmul(pz[:, t, :],
                                             negmT[0:1, c:c + P],
                                             csw[0:1, li - 1, :],
                                             start=False, stop=(not has_bias))
                if has_bias:
                    for t in range(T):
                        nc.tensor.matmul(pz[:, t, :], ones, bsc[:, li, :],
                                         start=False, stop=True)
                return pz

            def ln_stats(g, sums, pz_o, pz_g, pmt):
                """Row stats of raw-gelu g [P,2,T,H] (gelu sums in `sums`
                [P,S2], column c = s*T+t): returns rstd [P,S2] fp32; the
                folded path leaves the -m rows transposed in pmt[0:1, :].
                Non-fold path rescales g in place instead."""
                sq = mp.tile([P, S2], fp32, tag="sq")
                for s in range(2):
                    for t in range(T):
                        c = s * T + t
                        # scratch target: keeps the pz PSUM slot free after gelu
                        gsq = mp.tile([P, H], adt, tag="gsq")
                        nc.scalar.activation(gsq, g[:, s, t, :], AT.Square,
                                             accum_out=sq[:, c:c + 1])
                E = nc.gpsimd if ln_pool else nc.vector
                E.tensor_scalar_mul(sq, sq, 1.0 / H)
                m = mp.tile([P, S2], fp32, tag="m")
                mf = m
                E.tensor_scalar_mul(mf, sums, 1.0 / H)
                varb = mp.tile([P, S2], fp32, tag="varb")
                E.tensor_tensor(out=varb, in0=mf, in1=mf, op=OP.mult)
                E.tensor_tensor(out=varb, in0=sq, in1=varb, op=OP.subtract)
                E.tensor_scalar_add(varb, varb, LN_EPS)
                # rsqrt: quake seed on int ALU + `newton` NR iterations
                yi = mp.tile([P, S2], i32, tag="yi")
                E.tensor_scalar(
                    out=yi, in0=varb.bitcast(i32), scalar1=1,
                    scalar2=None, op0=OP.logical_shift_right)
                E.tensor_scalar(
                    out=yi, in0=yi, scalar1=-1, scalar2=0x5F3759DF,
                    op0=OP.mult, op1=OP.add)
                y = yi.bitcast(fp32)
                t1 = mp.tile([P, S2], fp32, tag="nt1")
                for _ in range(newton):
                    E.tensor_tensor(out=t1, in0=varb, in1=y, op=OP.mult)
                    E.tensor_tensor(out=t1, in0=t1, in1=y, op=OP.mult)
                    E.tensor_scalar(out=t1, in0=t1, scalar1=-0.5,
                                    scalar2=1.5, op0=OP.mult, op1=OP.add)
                    E.tensor_tensor(out=y, in0=y, in1=t1, op=OP.mult)
                if fold:
                    mh = mp.tile([P, S2], adt, tag="mh")
                    E.tensor_scalar(out=mh, in0=mf, scalar1=-1.0,
                                    scalar2=None, op0=OP.mult)
                    # all S2 -m rows transposed onto one partition-0 strip
                    for c in range(S2):
                        nc.tensor.transpose(pmt[0:1, c * P:(c + 1) * P],
                                            mh[:, c:c + 1], ident)
                    return y
                nmr = mp.tile([P, S2], fp32, tag="nmr")
                nc.vector.scalar_tensor_tensor(out=nmr, in0=mf, scalar=-1.0,
                                               in1=y, op0=OP.mult, op1=OP.mult)
                for s in range(2):
                    for t in range(T):
                        c = s * T + t
                        nc.scalar.activation(g[:, s, t, :], g[:, s, t, :],
                                             AT.Identity,
                                             bias=nmr[:, c:c + 1],
                                             scale=y[:, c:c + 1])
                return y

            def emit_sort_pass(p_idx, src_x, src_y, dst):
                """Emit bitonic pass p_idx over all T row-tiles. Pass 0 reads
                (src_x, src_y) [P,T,H]; later passes read src_x [P,T*1024]."""
                kind, L, d = _SCHED[p_idx]
                V = nc.vector
                if kind == "pair":
                    dv = dst.rearrange("p (t h g e) -> p t h g e",
                                       t=T, h=2, e=DPC)
                    for src, hh in ((src_x, 0), (src_y, 1)):
                        s = src.rearrange("p t (g e) -> p t g e", e=DPC)
                        V.tensor_tensor(out=dv[:, :, hh, :, 0::2],
                                        in0=s[:, :, :, 0::2],
                                        in1=s[:, :, :, 1::2], op=OP.min)
                        V.tensor_tensor(out=dv[:, :, hh, :, 1::2],
                                        in0=s[:, :, :, 0::2],
                                        in1=s[:, :, :, 1::2], op=OP.max)
                elif kind == "flip":
                    half = L // 2
                    s = src_x.rearrange("p (b e) -> p b e", e=L)
                    o = dst.rearrange("p (b e) -> p b e", e=L)
                    V.tensor_tensor(out=o[:, :, 0:half], in0=s[:, :, 0:half],
                                    in1=s[:, :, L - 1:half - 1:-1], op=OP.min)
                    V.tensor_tensor(out=o[:, :, half:L], in0=s[:, :, half:L],
                                    in1=s[:, :, half - 1::-1], op=OP.max)
                else:
                    s = src_x.rearrange("p (c e) -> p c e", e=2 * d)
                    o = dst.rearrange("p (c e) -> p c e", e=2 * d)
                    V.tensor_tensor(out=o[:, :, 0:d], in0=s[:, :, 0:d],
                                    in1=s[:, :, d:2 * d], op=OP.min)
                    V.tensor_tensor(out=o[:, :, d:2 * d], in0=s[:, :, 0:d],
                                    in1=s[:, :, d:2 * d], op=OP.max)

            # ---------------- pipeline stages
            def st_load(pipe, iv):
                xt = pipe.intermediate_tile([P, T, OBS], adt, name="xt")
                gt = pipe.intermediate_tile([P, T, OBS], adt, name="gt")
                nc.sync.dma_start(out=xt, in_=obs_v[iv])
                nc.sync.dma_start(out=gt, in_=gls_v[iv])
                return (xt, gt)

            def gelu_pair(pz_o, pz_g, g, rstd, sums):
                for s, pz in ((0, pz_o), (1, pz_g)):
                    for t in range(T):
                        c = s * T + t
                        kw = {}
                        if fold and rstd is not None:
                            kw["scale"] = rstd[:, c:c + 1]
                        nc.scalar.activation(g[:, s, t, :], pz[:, t, :],
                                             AT.Gelu_apprx_tanh,
                                             accum_out=sums[:, c:c + 1], **kw)

            def mk_layer(li):
                def st(pipe, iv, prev):
                    if li == 0:
                        srcs = prev
                        rstd_in = negmT_in = None
                    else:
                        g_in = prev[0]
                        rstd_in = prev[1] if len(prev) > 1 else None
                        negmT_in = prev[2] if len(prev) > 2 else None
                        srcs = (g_in[:, 0], g_in[:, 1])
                    g = pipe.intermediate_tile([P, 2, T, H], adt, name=f"g{li}")
                    if li >= n_layers:  # ablation: copy-through this mid layer
                        nc.scalar.copy(g, g_in)
                        return prev[:0] + (g,) + prev[1:]
                    pTf = ppt.tile([P, T * H], adt, tag="pT")
                    pz_o = matmuls(srcs[0], li, negmT_in, 0, pTf)
                    pz_g = matmuls(srcs[1], li, negmT_in, 1, pTf)
                    sums = mp.tile([P, S2], fp32, tag="sums")
                    gelu_pair(pz_o, pz_g, g, rstd_in, sums)
                    if ln_lite:
                        return (g,)
                    pmt = ppm.tile([1, S2 * P], adt, tag="pmt")
                    rstd = ln_stats(g, sums, pz_o, pz_g, pmt)
                    if fold:
                        negmT = pipe.intermediate_tile([1, S2 * P], adt,
                                                       name=f"nmT{li}")
                        nc.scalar.copy(negmT, pmt[0:1, :])
                        rs = pipe.intermediate_tile([P, S2], fp32,
                                                    name=f"rstd{li}")
                        nc.vector.tensor_copy(rs, rstd)
                        return (g, rs, negmT)
                    return (g,)
                return st

            def st_l3(pipe, iv, prev):
                g_in = prev[0]
                rstd_in = prev[1] if len(prev) > 1 else None
                negmT_in = prev[2] if len(prev) > 2 else None
                phis = pipe.intermediate_tile([P, T, H], sdt, name="phis")
                ypr = pipe.intermediate_tile([P, T, H], sdt, name="ypr")
                pTf = ppt.tile([P, T * H], adt, tag="pT")
                pz = matmuls(g_in[:, 0], 3, negmT_in, 0, pTf)
                for t in range(T):
                    if fold and rstd_in is not None:
                        nc.scalar.mul(phis[:, t, :], pz[:, t, :],
                                      rstd_in[:, t:t + 1])
                    else:
                        nc.scalar.copy(phis[:, t, :], pz[:, t, :])
                pzg = matmuls(g_in[:, 1], 3, negmT_in, 1, pTf)
                for t in range(T):
                    if fold and rstd_in is not None:
                        nc.vector.scalar_tensor_tensor(
                            out=ypr[:, t, :], in0=pzg[:, t, :],
                            scalar=rstd_in[:, T + t:T + t + 1],
                            in1=phis[:, t, :],
                            op0=OP.mult, op1=OP.max)
                    else:
                        nc.vector.tensor_tensor(out=ypr[:, t, :],
                                                in0=phis[:, t, :],
                                                in1=pzg[:, t, :], op=OP.max)
                return (phis, ypr)

            def st_sort_a(pipe, iv, prev):
                phis, ypr = prev
                bufA = pipe.intermediate_tile([P, T * 2 * H], sdt, name="bufA")
                bufB = pipe.intermediate_tile([P, T * 2 * H], sdt, name="bufB")
                emit_sort_pass(0, phis, ypr, bufA)
                cur, nxt = bufA, bufB
                for pidx in range(1, split_pass):
                    if pidx < n_passes:
                        emit_sort_pass(pidx, cur, None, nxt)
                    cur, nxt = nxt, cur
                return (bufA, bufB)

            def st_sort_b(pipe, iv, prev):
                bufA, bufB = prev
                cur, nxt = (bufB, bufA) if split_pass % 2 == 0 else (bufA, bufB)
                for pidx in range(split_pass, 15):
                    if pidx < n_passes:
                        emit_sort_pass(pidx, cur, None, nxt)
                    cur, nxt = nxt, cur
                fin = cur
                fv = fin.rearrange("p (t h g e) -> p t h g e", t=T, h=2, e=DPC)
                TE = nc.gpsimd if tail_pool else nc.vector
                # coupling: u[i] <- max(u[i], v[i-1]) for i>=1, in place
                for t in range(T):
                    TE.tensor_tensor(out=fv[:, t, 0, :, 1:DPC],
                                     in0=fv[:, t, 0, :, 1:DPC],
                                     in1=fv[:, t, 1, :, 0:DPC - 1],
                                     op=OP.max)
                red = sp.tile([P, T, 2, NCOMP], fp32, tag="red")
                nc.vector.tensor_reduce(
                    out=red.rearrange("p t h g -> p (t h g)"),
                    in_=fin.rearrange("p (a e) -> p a e", e=DPC),
                    axis=mybir.AxisListType.X, op=OP.add)
                comp = sp.tile([P, T, NCOMP], fp32, tag="comp")
                nc.vector.tensor_tensor(out=comp, in0=red[:, :, 1, :],
                                        in1=red[:, :, 0, :], op=OP.subtract)
                cs = sp.tile([P, T], fp32, tag="cs")
                nc.vector.tensor_reduce(out=cs, in_=comp,
                                        axis=mybir.AxisListType.X, op=OP.add)
                cm = sp.tile([P, T], fp32, tag="cm")
                nc.vector.tensor_reduce(out=cm, in_=comp,
                                        axis=mybir.AxisListType.X, op=OP.max)
                res = sp.tile([P, T], fp32, tag="res")
                TE.tensor_scalar(out=res, in0=cs, scalar1=avec[:, 0:1],
                                 scalar2=None, op0=OP.mult)
                TE.scalar_tensor_tensor(out=res, in0=cm,
                                        scalar=avec[:, 1:2], in1=res,
                                        op0=OP.mult, op1=OP.add)
                nc.sync.dma_start(out=out_v[iv], in_=res)

            stages = [st_load, mk_layer(0), mk_layer(1), mk_layer(2), st_l3,
                      st_sort_a, st_sort_b]

            def run_pipe():
                he = (mybir.EngineType.PE, mybir.EngineType.DVE,
                      mybir.EngineType.Activation, mybir.EngineType.SP,
                      mybir.EngineType.Pool) if hints else ()
                tc.For_i_pipelined(stages, 0, nt, 1, pool=pipe_pool,
                                   unroll=unroll, staged_num_bufs=stage_bufs,
                                   hint_engines=he)

            if repeats == 1:
                run_pipe()
            else:
                with tc.For_i(0, repeats, 1):
                    run_pipe()

    nc.finalize()
    return nc


# ---------------------------------------------------------------- host wrapper
def _prep_host(inputs, mm16=True):
    """Fold LN affine params into the following layer's weights; build avec."""
    f32 = np.float32
    adt = np.float16 if mm16 else f32
    W0 = np.asarray(inputs["W0"], f32)
    b0 = np.asarray(inputs["b0"], f32)
    w, b = [W0], [b0]
    for i in (0, 1, 2):
        s = np.asarray(inputs[f"ln{i}_s"], f32)
        t = np.asarray(inputs[f"ln{i}_b"], f32)
        Wn = np.asarray(inputs[("W1", "W2", "W3")[i]], f32)
        bn = np.asarray(inputs[("b1", "b2", "b3")[i]], f32)
        w.append(s[:, None] * Wn)
        b.append(bn + t @ Wn)
    bs = np.stack(b, 0)  # [4, 512]
    has_bias = bool(np.any(np.abs(bs) > 0))
    csw = np.stack([w[1].sum(0), w[2].sum(0), w[3].sum(0)], 0)  # [3, 512]
    alpha = float(np.asarray(inputs["alpha"]))
    a = 1.0 / (1.0 + np.exp(-alpha))
    avec = np.empty((P, 2), f32)
    avec[:, 0] = a / NCOMP
    avec[:, 1] = 1.0 - a
    return ([x.astype(adt) for x in w], bs.astype(adt), csw.astype(adt),
            avec, has_bias)


def _probe_devices():
    """Poke every core with a tiny op; retries to shake off a stale
    NRT_EXEC_UNIT_UNRECOVERABLE state left by a previous process."""
    import jax
    import jax.numpy as jnp

    for attempt in range(3):
        try:
            for d in jax.devices()[:NCORES]:
                jnp.zeros((1,), jnp.float32, device=d).block_until_ready()
            return
        except Exception:
            if attempt == 2:
                raise


def run_on_device(inputs, rows_total=B, trace=False, repeats=1, **build_kw):
    """Shard, run on 8 cores, gather. Returns (out [rows_total], results obj)."""
    from concourse.bass_utils import run_bass_kernel_spmd

    _probe_devices()

    mm16 = bool(build_kw.get("mm16", 1))
    (w0, w1, w2, w3), bs, csw, avec, has_bias = _prep_host(inputs, mm16=mm16)
    build_kw.setdefault("has_bias", int(has_bias))

    rows_core = rows_total // NCORES
    key = (rows_core, repeats, tuple(sorted(build_kw.items())))
    if key not in _CACHE:
        _CACHE[key] = build_nc(rows_core, repeats=repeats, **build_kw)
    nc = _CACHE[key]

    adt = np.float16 if mm16 else np.float32
    ob = np.ascontiguousarray(
        np.asarray(inputs["observations"])[:rows_total].astype(adt))
    gl = np.ascontiguousarray(
        np.asarray(inputs["goals"])[:rows_total].astype(adt))
    in_maps = []
    for c in range(NCORES):
        sl = slice(c * rows_core, (c + 1) * rows_core)
        m = {
            "observations": ob[sl], "goals": gl[sl],
            "w0": w0, "w1": w1, "w2": w2, "w3": w3, "csw": csw, "avec": avec,
        }
        if has_bias:
            m["bs"] = bs
        in_maps.append(m)
    r = run_bass_kernel_spmd(nc, in_maps, list(range(NCORES)), trace=trace)
    outp = np.concatenate([r.results[c]["out"] for c in range(NCORES)])
    return outp, r


def kernel(**inputs):
    out, _ = run_on_device(inputs)
    return out.astype(np.float32)
